# revision 9
# baseline (speedup 1.0000x reference)
import numpy as np
from contextlib import ExitStack

import concourse.bass as bass
import concourse.bacc as bacc
import concourse.tile as tile
from concourse import mybir
from concourse.bass_utils import run_bass_kernel_spmd

F32 = mybir.dt.float32
ALU = mybir.AluOpType
AF = mybir.ActivationFunctionType
AX = mybir.AxisListType

R = 131072          # total rays
S = 128             # samples per ray
NCORES = 8
RPC = R // NCORES   # 16384 rays per core
NT = 8              # tiles per core
SEG = 16            # rays (segments) per partition row per tile
LAM = 1e-3

_NC_CACHE = {}
LAST_RESULTS = None


def _build_nc():
    nc = bacc.Bacc("TRN2", target_bir_lowering=False, debug=False,
                   num_devices=NCORES)

    # packed inputs: one DMA per consumer group so no instruction waits on
    # two DMA producers (walrus sync-wait limit)
    smp_d = nc.dram_tensor("smp", [NT, 128, 3, SEG, S], F32,
                           kind="ExternalInput").ap()
    rgbo_d = nc.dram_tensor("rgbo", [128, 896], F32, kind="ExternalInput").ap()

    rgbl_d = nc.dram_tensor("rgb_loss", [128, 384], F32, kind="ExternalOutput").ap()
    opal_d = nc.dram_tensor("opa_loss", [128, 128], F32, kind="ExternalOutput").ap()
    dist_d = nc.dram_tensor("dist_loss", [NT, 128, SEG], F32,
                            kind="ExternalOutput").ap()

    with tile.TileContext(nc) as tc:
        with ExitStack() as ctx:
            const_pool = ctx.enter_context(tc.tile_pool(name="const", bufs=1))
            inp_pool = ctx.enter_context(tc.tile_pool(name="inp", bufs=2))
            work_pool = ctx.enter_context(tc.tile_pool(name="work", bufs=2))
            small_pool = ctx.enter_context(tc.tile_pool(name="small", bufs=2))
            rgb_pool = ctx.enter_context(tc.tile_pool(name="rgb", bufs=1))

            # segmented-scan mask: 1 everywhere except 0 at sample 0 of each segment
            mask = const_pool.tile([128, SEG, S], F32)
            mask2 = mask.rearrange("p a b -> p (a b)")
            nc.vector.memset(mask2, 1.0)
            nc.vector.memset(mask[:, :, 0], 0.0)

            eps_bias = const_pool.tile([128, 1], F32)
            nc.vector.memset(eps_bias[:], 1e-10)

            # ---- rgb + opacity losses (once per core) ----
            rgbo = rgb_pool.tile([128, 896], F32)
            rl = rgb_pool.tile([128, 384], F32)
            ol = rgb_pool.tile([128, 128], F32)
            lno = rgb_pool.tile([128, 128], F32)
            t1 = rgb_pool.tile([128, 128], F32)
            diff = rgb_pool.tile([128, 384], F32)

            nc.sync.dma_start(rgbo[:], rgbo_d)
            rp = rgbo[:, 0:384]
            rg = rgbo[:, 384:768]
            op = rgbo[:, 768:896]

            # ln(o + 1e-10) first so the ACT Ln table loads before all the Squares
            nc.scalar.activation(lno[:], op, AF.Ln, bias=eps_bias[:], scale=1.0)
            nc.vector.tensor_sub(diff[:], rp, rg)
            nc.scalar.activation(rl[:], diff[:], AF.Square, bias=0.0, scale=1.0)
            # t1 = (op + 1e-10) * lno
            nc.vector.scalar_tensor_tensor(t1[:], op, 1e-10, lno[:],
                                           ALU.add, ALU.mult)
            # ol = -LAM * t1
            nc.vector.tensor_scalar_mul(ol[:], t1[:], -LAM)

            nc.sync.dma_start(rgbl_d, rl[:])
            nc.sync.dma_start(opal_d, ol[:])

            # ---- distortion loss: NT tiles of [128, 3, SEG, S] ----
            for i in range(NT):
                smp = inp_pool.tile([128, 3, SEG, S], F32, tag="smp")
                nc.sync.dma_start(smp[:], smp_d[i])

                w3 = smp[:, 0]
                t3 = smp[:, 1]
                d3 = smp[:, 2]
                w2 = w3.rearrange("p a b -> p (a b)")
                t2 = t3.rearrange("p a b -> p (a b)")
                d2 = d3.rearrange("p a b -> p (a b)")

                icw3 = work_pool.tile([128, SEG, S], F32, tag="icw")
                wt3 = work_pool.tile([128, SEG, S], F32, tag="wt")
                sq3 = work_pool.tile([128, SEG, S], F32, tag="sq")
                v3 = work_pool.tile([128, SEG, S], F32, tag="v")
                x3 = work_pool.tile([128, SEG, S], F32, tag="x")
                u3 = work_pool.tile([128, SEG, S], F32, tag="u")
                icw2 = icw3.rearrange("p a b -> p (a b)")
                wt2 = wt3.rearrange("p a b -> p (a b)")
                sq2 = sq3.rearrange("p a b -> p (a b)")
                v2 = v3.rearrange("p a b -> p (a b)")
                x2 = x3.rearrange("p a b -> p (a b)")
                u2 = u3.rearrange("p a b -> p (a b)")

                # segmented inclusive cumsum of w: state = state*mask + w
                nc.vector.tensor_tensor_scan(icw2, mask2, w2, 0.0,
                                             ALU.mult, ALU.add)
                # wt = w * t    (GpSimd)
                nc.gpsimd.tensor_mul(wt2, w2, t2)
                # sq = w^2      (ACT)
                nc.scalar.activation(sq2, w2, AF.Square, bias=0.0, scale=1.0)
                # v = 2*icw - w (DVE)
                nc.vector.scalar_tensor_tensor(v2, icw2, 2.0, w2,
                                               ALU.mult, ALU.subtract)
                # x = v * wt    (GpSimd)
                nc.gpsimd.tensor_mul(x2, v2, wt2)
                # u = sq * d    (GpSimd)
                nc.gpsimd.tensor_mul(u2, sq2, d2)

                r1 = small_pool.tile([128, SEG], F32, tag="r1")
                r2 = small_pool.tile([128, SEG], F32, tag="r2")
                r3 = small_pool.tile([128, SEG], F32, tag="r3")
                up = small_pool.tile([128, SEG], F32, tag="up")
                c1 = small_pool.tile([128, SEG], F32, tag="c1")
                dl = small_pool.tile([128, SEG], F32, tag="dl")

                nc.vector.tensor_reduce(r1[:], x3[:], AX.X, ALU.add)
                nc.vector.tensor_reduce(r2[:], u3[:], AX.X, ALU.add)
                nc.vector.tensor_reduce(r3[:], wt3[:], AX.X, ALU.add)

                # dist = 2*LAM * (r1 + r2/6 - sw*swt); sw = icw at segment end
                # up = (sw * -2LAM) * r3
                nc.vector.scalar_tensor_tensor(up[:], icw3[:, :, S - 1], -2.0 * LAM,
                                               r3[:], ALU.mult, ALU.mult)
                # c1 = (r2 * 1/6) + r1
                nc.vector.scalar_tensor_tensor(c1[:], r2[:], 1.0 / 6.0, r1[:],
                                               ALU.mult, ALU.add)
                # dl = (c1 * 2LAM) + up
                nc.vector.scalar_tensor_tensor(dl[:], c1[:], 2.0 * LAM, up[:],
                                               ALU.mult, ALU.add)

                nc.sync.dma_start(dist_d[i], dl[:])

    nc.compile()
    return nc


def _selfcheck(rgb_pred, rgb_gt, opacity, ws, deltas, ts,
               rgb_loss, opa_loss, dist_loss):
    idx = np.linspace(0, R - 1, 128).astype(np.int64)
    rp = rgb_pred.reshape(R, 3)[idx].astype(np.float64)
    rg = rgb_gt.reshape(R, 3)[idx].astype(np.float64)
    e_rgb = (rp - rg) ** 2
    o = opacity.reshape(R)[idx].astype(np.float64) + 1e-10
    e_opa = -LAM * o * np.log(o)
    w = ws.reshape(R, S)[idx].astype(np.float64)
    t = ts.reshape(R, S)[idx].astype(np.float64)
    d = deltas.reshape(R, S)[idx].astype(np.float64)
    icw = np.cumsum(w, axis=1)
    wt = w * t
    r1 = np.sum(wt * (2.0 * icw - w), axis=1)
    r2 = np.sum(w * w * d, axis=1)
    r3 = np.sum(wt, axis=1)
    e_dist = 2.0 * LAM * (r1 + r2 / 6.0 - icw[:, -1] * r3)

    def rel(e, a):
        return np.max(np.abs(e - a)) / (np.max(np.abs(e)) + 1e-30)

    return max(rel(e_rgb, rgb_loss[idx].astype(np.float64)),
               rel(e_opa, opa_loss.reshape(R)[idx].astype(np.float64)),
               rel(e_dist, dist_loss[idx].astype(np.float64)))


def kernel(rgb_pred, rgb_gt, opacity, ws, deltas, ts, rays_a):
    global LAST_RESULTS
    if "nc" not in _NC_CACHE:
        _NC_CACHE["nc"] = _build_nc()
    nc = _NC_CACHE["nc"]

    rgb_pred = np.ascontiguousarray(rgb_pred, dtype=np.float32)
    rgb_gt = np.ascontiguousarray(rgb_gt, dtype=np.float32)
    opacity = np.ascontiguousarray(opacity, dtype=np.float32)
    ws = np.ascontiguousarray(ws, dtype=np.float32)
    deltas = np.ascontiguousarray(deltas, dtype=np.float32)
    ts = np.ascontiguousarray(ts, dtype=np.float32)

    w5 = ws.reshape(NCORES, NT, 128, SEG, S)
    t5 = ts.reshape(NCORES, NT, 128, SEG, S)
    d5 = deltas.reshape(NCORES, NT, 128, SEG, S)
    smp = np.empty((NCORES, NT, 128, 3, SEG, S), dtype=np.float32)
    smp[:, :, :, 0] = w5
    smp[:, :, :, 1] = t5
    smp[:, :, :, 2] = d5

    rgbo = np.concatenate([
        rgb_pred.reshape(NCORES, 128, 384),
        rgb_gt.reshape(NCORES, 128, 384),
        opacity.reshape(NCORES, 128, 128),
    ], axis=2)

    in_maps = []
    for c in range(NCORES):
        in_maps.append({"smp": smp[c], "rgbo": rgbo[c]})

    for attempt in range(2):
        res = run_bass_kernel_spmd(nc, in_maps, list(range(NCORES)))
        LAST_RESULTS = res

        rgb_loss = np.concatenate(
            [res.results[c]["rgb_loss"].reshape(RPC, 3) for c in range(NCORES)],
            axis=0)
        opa_loss = np.concatenate(
            [res.results[c]["opa_loss"].reshape(RPC, 1) for c in range(NCORES)],
            axis=0)
        dist_loss = np.concatenate(
            [res.results[c]["dist_loss"].reshape(RPC) for c in range(NCORES)],
            axis=0)
        err = _selfcheck(rgb_pred, rgb_gt, opacity, ws, deltas, ts,
                         rgb_loss, opa_loss, dist_loss)
        if err < 1e-3:
            break
    return (rgb_loss, opa_loss, dist_loss)


# revision 10
# speedup vs baseline: 1.9402x; 1.9402x over previous
import numpy as np
from operator import add
from contextlib import ExitStack

import concourse.bass as bass
import concourse.bacc as bacc
import concourse.tile as tile
from concourse import mybir
from concourse.bass_utils import run_bass_kernel_spmd
import concourse.dve_ops as dve_ops
from concourse.dve_spec import AluOp, Spec, Src0, Src1, scan, lower, _has_src1
from concourse.dve_uop import DveOpSpec

F32 = mybir.dt.float32
ALU = mybir.AluOpType
AF = mybir.ActivationFunctionType
AX = mybir.AxisListType

R = 131072          # total rays
S = 128             # samples per ray
NCORES = 8
RPC = R // NCORES   # 16384 rays per core
NT = 8              # tiles per core
SEG = 16            # rays (segments) per partition row per tile
LAM = 1e-3

_NC_CACHE = {}
LAST_RESULTS = None


def _register_op(name, spec, subdim=False):
    if name in dve_ops._SUB_OPCODE_FOR_NAME:
        for op in dve_ops.OPS:
            if op.name == name:
                return op
    row = dve_ops._CUSTOM_DVE_ROW_BASE + len(dve_ops.OPS)
    assert row < 0x20
    op = dve_ops.DveOp(name, spec, subdim, uops_sha={})
    dve_ops.OPS.append(op)
    dve_ops._SUB_OPCODE_FOR_NAME[name] = row
    dve_ops.CUSTOM_DVE_SPECS[name] = spec
    for ver in ("v3", "v4"):
        s = DveOpSpec(name=name, opcode=row, uops=lower(spec, ver=ver),
                      rd1_en=_has_src1(spec))
        op.uops_sha[ver] = s.sha(ver)
    return op


def _ref_r1(in0, in1, s0, s1, imm2):
    w = in0.astype(np.float32)
    t = in1.astype(np.float32)
    icw = np.cumsum(w, -1, dtype=np.float32)
    icwt = np.cumsum((w * t).astype(np.float32), -1, dtype=np.float32)
    b = (w * (t * icw - icwt)).astype(np.float32)
    return b, b.reshape(b.shape[0], -1).sum(-1, keepdims=True)


# out = w*(t*cumsum(w) - cumsum(w*t)); accum_out = per-row sum
# (= sum_{i>j} w_i w_j (t_i - t_j) per segment when one segment per row)
NERF_R1 = _register_op(
    "NERF_R1_ANT",
    Spec(
        body=Src0 * (Src1 * scan(AluOp.ADD, Src0)
                     - scan(AluOp.ADD, Src0 * Src1)),
        accum=add,
        reference=_ref_r1,
    ),
)


def _build_nc():
    nc = bacc.Bacc("TRN2", target_bir_lowering=False, debug=False,
                   num_devices=NCORES)

    # packed inputs: one DMA per consumer group so no instruction waits on
    # two DMA producers (walrus sync-wait limit)
    smp_d = nc.dram_tensor("smp", [NT, 128, 3, SEG, S], F32,
                           kind="ExternalInput").ap()
    rgbo_d = nc.dram_tensor("rgbo", [128, 896], F32, kind="ExternalInput").ap()

    rgbl_d = nc.dram_tensor("rgb_loss", [128, 384], F32, kind="ExternalOutput").ap()
    opal_d = nc.dram_tensor("opa_loss", [128, 128], F32, kind="ExternalOutput").ap()
    dist_d = nc.dram_tensor("dist_loss", [NT, 128, SEG], F32,
                            kind="ExternalOutput").ap()

    with tile.TileContext(nc) as tc:
        with ExitStack() as ctx:
            const_pool = ctx.enter_context(tc.tile_pool(name="const", bufs=1))
            inp_pool = ctx.enter_context(tc.tile_pool(name="inp", bufs=2))
            work_pool = ctx.enter_context(tc.tile_pool(name="work", bufs=2))
            small_pool = ctx.enter_context(tc.tile_pool(name="small", bufs=2))
            rgb_pool = ctx.enter_context(tc.tile_pool(name="rgb", bufs=1))

            eps_bias = const_pool.tile([128, 1], F32)
            nc.vector.memset(eps_bias[:], 1e-10)

            # ---- rgb + opacity losses (once per core) ----
            rgbo = rgb_pool.tile([128, 896], F32)
            rl = rgb_pool.tile([128, 384], F32)
            ol = rgb_pool.tile([128, 128], F32)
            lno = rgb_pool.tile([128, 128], F32)
            t1 = rgb_pool.tile([128, 128], F32)
            diff = rgb_pool.tile([128, 384], F32)

            nc.sync.dma_start(rgbo[:], rgbo_d)
            rp = rgbo[:, 0:384]
            rg = rgbo[:, 384:768]
            op = rgbo[:, 768:896]

            # ln(o + 1e-10) first so the ACT Ln table loads before the Squares
            nc.scalar.activation(lno[:], op, AF.Ln, bias=eps_bias[:], scale=1.0)
            nc.vector.tensor_sub(diff[:], rp, rg)
            nc.scalar.activation(rl[:], diff[:], AF.Square, bias=0.0, scale=1.0)
            # t1 = (op + 1e-10) * lno
            nc.vector.scalar_tensor_tensor(t1[:], op, 1e-10, lno[:],
                                           ALU.add, ALU.mult)
            # ol = -LAM * t1
            nc.vector.tensor_scalar_mul(ol[:], t1[:], -LAM)

            nc.sync.dma_start(rgbl_d, rl[:])
            nc.sync.dma_start(opal_d, ol[:])

            # ---- distortion loss: NT tiles of [128, 3, SEG, S] ----
            for i in range(NT):
                smp = inp_pool.tile([128, 3, SEG, S], F32, tag="smp")
                nc.sync.dma_start(smp[:], smp_d[i])

                w3 = smp[:, 0]
                t3 = smp[:, 1]
                d3 = smp[:, 2]
                w2 = w3.rearrange("p a b -> p (a b)")
                d2 = d3.rearrange("p a b -> p (a b)")

                r1 = small_pool.tile([128, SEG], F32, tag="r1")
                r2 = small_pool.tile([128, SEG], F32, tag="r2")
                c1 = small_pool.tile([128, SEG], F32, tag="c1")
                dl = small_pool.tile([128, SEG], F32, tag="dl")

                # r1 per segment via fused custom DVE op (one seg per row)
                for s in range(SEG):
                    scr = work_pool.tile([128, S], F32, tag=f"scr{s % 2}")
                    nc.vector._custom_dve(
                        NERF_R1, out=scr[:],
                        in0=w3[:, s, :], in1=t3[:, s, :],
                        accum_out=r1[:, s:s + 1])

                # r2 = sum(w^2 * d) per segment
                sq3 = work_pool.tile([128, SEG, S], F32, tag="sq")
                u3 = work_pool.tile([128, SEG, S], F32, tag="u")
                sq2 = sq3.rearrange("p a b -> p (a b)")
                u2 = u3.rearrange("p a b -> p (a b)")
                nc.scalar.activation(sq2, w2, AF.Square, bias=0.0, scale=1.0)
                nc.gpsimd.tensor_mul(u2, sq2, d2)
                nc.vector.tensor_reduce(r2[:], u3[:], AX.X, ALU.add)

                # dist = 2*LAM * (r1 + r2/6)
                nc.vector.scalar_tensor_tensor(c1[:], r2[:], 1.0 / 6.0, r1[:],
                                               ALU.mult, ALU.add)
                nc.vector.tensor_scalar_mul(dl[:], c1[:], 2.0 * LAM)

                nc.sync.dma_start(dist_d[i], dl[:])

    nc.compile()
    return nc


def _selfcheck(rgb_pred, rgb_gt, opacity, ws, deltas, ts,
               rgb_loss, opa_loss, dist_loss):
    idx = np.linspace(0, R - 1, 128).astype(np.int64)
    rp = rgb_pred.reshape(R, 3)[idx].astype(np.float64)
    rg = rgb_gt.reshape(R, 3)[idx].astype(np.float64)
    e_rgb = (rp - rg) ** 2
    o = opacity.reshape(R)[idx].astype(np.float64) + 1e-10
    e_opa = -LAM * o * np.log(o)
    w = ws.reshape(R, S)[idx].astype(np.float64)
    t = ts.reshape(R, S)[idx].astype(np.float64)
    d = deltas.reshape(R, S)[idx].astype(np.float64)
    icw = np.cumsum(w, axis=1)
    wt = w * t
    r1 = np.sum(wt * (2.0 * icw - w), axis=1)
    r2 = np.sum(w * w * d, axis=1)
    r3 = np.sum(wt, axis=1)
    e_dist = 2.0 * LAM * (r1 + r2 / 6.0 - icw[:, -1] * r3)

    def rel(e, a):
        return np.max(np.abs(e - a)) / (np.max(np.abs(e)) + 1e-30)

    return max(rel(e_rgb, rgb_loss[idx].astype(np.float64)),
               rel(e_opa, opa_loss.reshape(R)[idx].astype(np.float64)),
               rel(e_dist, dist_loss[idx].astype(np.float64)))


def kernel(rgb_pred, rgb_gt, opacity, ws, deltas, ts, rays_a):
    global LAST_RESULTS
    if "nc" not in _NC_CACHE:
        _NC_CACHE["nc"] = _build_nc()
    nc = _NC_CACHE["nc"]

    rgb_pred = np.ascontiguousarray(rgb_pred, dtype=np.float32)
    rgb_gt = np.ascontiguousarray(rgb_gt, dtype=np.float32)
    opacity = np.ascontiguousarray(opacity, dtype=np.float32)
    ws = np.ascontiguousarray(ws, dtype=np.float32)
    deltas = np.ascontiguousarray(deltas, dtype=np.float32)
    ts = np.ascontiguousarray(ts, dtype=np.float32)

    w5 = ws.reshape(NCORES, NT, 128, SEG, S)
    t5 = ts.reshape(NCORES, NT, 128, SEG, S)
    d5 = deltas.reshape(NCORES, NT, 128, SEG, S)
    smp = np.empty((NCORES, NT, 128, 3, SEG, S), dtype=np.float32)
    smp[:, :, :, 0] = w5
    smp[:, :, :, 1] = t5
    smp[:, :, :, 2] = d5

    rgbo = np.concatenate([
        rgb_pred.reshape(NCORES, 128, 384),
        rgb_gt.reshape(NCORES, 128, 384),
        opacity.reshape(NCORES, 128, 128),
    ], axis=2)

    in_maps = []
    for c in range(NCORES):
        in_maps.append({"smp": smp[c], "rgbo": rgbo[c]})

    for attempt in range(2):
        res = run_bass_kernel_spmd(nc, in_maps, list(range(NCORES)))
        LAST_RESULTS = res

        rgb_loss = np.concatenate(
            [res.results[c]["rgb_loss"].reshape(RPC, 3) for c in range(NCORES)],
            axis=0)
        opa_loss = np.concatenate(
            [res.results[c]["opa_loss"].reshape(RPC, 1) for c in range(NCORES)],
            axis=0)
        dist_loss = np.concatenate(
            [res.results[c]["dist_loss"].reshape(RPC) for c in range(NCORES)],
            axis=0)
        err = _selfcheck(rgb_pred, rgb_gt, opacity, ws, deltas, ts,
                         rgb_loss, opa_loss, dist_loss)
        if err < 1e-3:
            break
    return (rgb_loss, opa_loss, dist_loss)


# revision 17
# speedup vs baseline: 2.1603x; 1.1134x over previous
import numpy as np
from operator import add
from contextlib import ExitStack

import concourse.bass as bass
import concourse.bacc as bacc
import concourse.tile as tile
from concourse import mybir
from concourse.bass_utils import run_bass_kernel_spmd
import concourse.dve_ops as dve_ops
from concourse.dve_spec import AluOp, Spec, Src0, Src1, scan, lower, _has_src1
from concourse.dve_uop import DveOpSpec

F32 = mybir.dt.float32
ALU = mybir.AluOpType
AF = mybir.ActivationFunctionType
AX = mybir.AxisListType

R = 131072          # total rays
S = 128             # samples per ray
NCORES = 8
RPC = R // NCORES   # 16384 rays per core
NT = 8              # tiles per core
SEG = 16            # rays (segments) per partition row per tile
LAM = 1e-3

_NC_CACHE = {}
LAST_RESULTS = None


def _register_op(name, spec, subdim=False):
    if name in dve_ops._SUB_OPCODE_FOR_NAME:
        for op in dve_ops.OPS:
            if op.name == name:
                return op
    row = dve_ops._CUSTOM_DVE_ROW_BASE + len(dve_ops.OPS)
    assert row < 0x20
    op = dve_ops.DveOp(name, spec, subdim, uops_sha={})
    dve_ops.OPS.append(op)
    dve_ops._SUB_OPCODE_FOR_NAME[name] = row
    dve_ops.CUSTOM_DVE_SPECS[name] = spec
    for ver in ("v3", "v4"):
        s = DveOpSpec(name=name, opcode=row, uops=lower(spec, ver=ver),
                      rd1_en=_has_src1(spec))
        op.uops_sha[ver] = s.sha(ver)
    return op


def _ref_r1(in0, in1, s0, s1, imm2):
    w = in0.astype(np.float32)
    t = in1.astype(np.float32)
    icw = np.cumsum(w, -1, dtype=np.float32)
    icwt = np.cumsum((w * t).astype(np.float32), -1, dtype=np.float32)
    b = (w * (t * icw - icwt)).astype(np.float32)
    return b, b.reshape(b.shape[0], -1).sum(-1, keepdims=True)


# out = w*(t*cumsum(w) - cumsum(w*t)); accum_out = per-row sum
# (= sum_{i>j} w_i w_j (t_i - t_j) per segment when one segment per row)
NERF_R1 = _register_op(
    "NERF_R1_ANT",
    Spec(
        body=Src0 * (Src1 * scan(AluOp.ADD, Src0)
                     - scan(AluOp.ADD, Src0 * Src1)),
        accum=add,
        reference=_ref_r1,
    ),
)


def _build_nc():
    nc = bacc.Bacc("TRN2", target_bir_lowering=False, debug=False,
                   num_devices=NCORES)

    # packed inputs: one DMA per consumer group so no instruction waits on
    # two DMA producers (walrus sync-wait limit)
    smp_d = nc.dram_tensor("smp", [NT, 128, 3, SEG, S], F32,
                           kind="ExternalInput").ap()
    rgbo_d = nc.dram_tensor("rgbo", [128, 896], F32, kind="ExternalInput").ap()

    rgbl_d = nc.dram_tensor("rgb_loss", [128, 384], F32, kind="ExternalOutput").ap()
    opal_d = nc.dram_tensor("opa_loss", [128, 128], F32, kind="ExternalOutput").ap()
    dist_d = nc.dram_tensor("dist_loss", [NT, 128, SEG], F32,
                            kind="ExternalOutput").ap()

    with tile.TileContext(nc) as tc:
        with ExitStack() as ctx:
            const_pool = ctx.enter_context(tc.tile_pool(name="const", bufs=1))
            inp_pool = ctx.enter_context(tc.tile_pool(name="inp", bufs=3))
            work_pool = ctx.enter_context(tc.tile_pool(name="work", bufs=2))
            small_pool = ctx.enter_context(tc.tile_pool(name="small", bufs=2))
            rgb_pool = ctx.enter_context(tc.tile_pool(name="rgb", bufs=1))

            eps_bias = const_pool.tile([128, 1], F32)
            nc.vector.memset(eps_bias[:], 1e-10)

            # ---- rgb + opacity losses (once per core) ----
            rgbo = rgb_pool.tile([128, 896], F32)
            rl = rgb_pool.tile([128, 384], F32)
            ol = rgb_pool.tile([128, 128], F32)
            lno = rgb_pool.tile([128, 128], F32)
            t1 = rgb_pool.tile([128, 128], F32)
            diff = rgb_pool.tile([128, 384], F32)

            nc.sync.dma_start(rgbo[:], rgbo_d)
            rp = rgbo[:, 0:384]
            rg = rgbo[:, 384:768]
            op = rgbo[:, 768:896]

            # ln(o + 1e-10) first so the ACT Ln table loads before the Squares
            nc.scalar.activation(lno[:], op, AF.Ln, bias=eps_bias[:], scale=1.0)
            nc.gpsimd.tensor_sub(diff[:], rp, rg)
            nc.scalar.activation(rl[:], diff[:], AF.Square, bias=0.0, scale=1.0)
            # t1 = (op + 1e-10) * lno
            nc.vector.scalar_tensor_tensor(t1[:], op, 1e-10, lno[:],
                                           ALU.add, ALU.mult)
            # ol = -LAM * t1
            nc.vector.tensor_scalar_mul(ol[:], t1[:], -LAM)

            nc.sync.dma_start(rgbl_d, rl[:])
            nc.sync.dma_start(opal_d, ol[:])

            # ---- distortion loss: NT tiles of [128, 3, SEG, S] ----
            for i in range(NT):
                smp = inp_pool.tile([128, 3, SEG, S], F32, tag="smp")
                if i % 2 == 0:
                    nc.sync.dma_start(smp[:], smp_d[i])
                else:
                    nc.scalar.dma_start(smp[:], smp_d[i])

                w3 = smp[:, 0]
                t3 = smp[:, 1]
                d3 = smp[:, 2]
                w2 = w3.rearrange("p a b -> p (a b)")
                d2 = d3.rearrange("p a b -> p (a b)")

                r1 = small_pool.tile([128, SEG], F32, tag="r1")
                r2c = small_pool.tile([128, SEG], F32, tag="r2c")
                dl = small_pool.tile([128, SEG], F32, tag="dl")

                # w is pre-scaled by sqrt(2*LAM) on host, so accum = 2*LAM*r1
                for s in range(SEG):
                    scr = work_pool.tile([128, S], F32, tag=f"scr{s % 2}")
                    nc.vector._custom_dve(
                        NERF_R1, out=scr[:],
                        in0=w3[:, s, :], in1=t3[:, s, :],
                        accum_out=r1[:, s:s + 1])

                # sq = (w/sqrt(6))^2 = 2*LAM*w_orig^2/6; u = sq*d; r2c = sum(u)
                sq3 = work_pool.tile([128, SEG, S], F32, tag="sq")
                u3 = work_pool.tile([128, SEG, S], F32, tag="u")
                sq2 = sq3.rearrange("p a b -> p (a b)")
                u2 = u3.rearrange("p a b -> p (a b)")
                nc.scalar.activation(sq2, w2, AF.Square, bias=0.0,
                                     scale=0.4082482904638631)
                nc.gpsimd.tensor_mul(u2, sq2, d2)
                nc.vector.tensor_reduce(r2c[:], u3[:], AX.X, ALU.add)

                # dl = 2*LAM*(r1 + r2/6), both addends pre-scaled
                nc.gpsimd.tensor_add(dl[:], r1[:], r2c[:])

                nc.sync.dma_start(dist_d[i], dl[:])

    nc.compile()
    return nc


def _selfcheck(rgb_pred, rgb_gt, opacity, ws, deltas, ts,
               rgb_loss, opa_loss, dist_loss):
    idx = np.linspace(0, R - 1, 128).astype(np.int64)
    rp = rgb_pred.reshape(R, 3)[idx].astype(np.float64)
    rg = rgb_gt.reshape(R, 3)[idx].astype(np.float64)
    e_rgb = (rp - rg) ** 2
    o = opacity.reshape(R)[idx].astype(np.float64) + 1e-10
    e_opa = -LAM * o * np.log(o)
    w = ws.reshape(R, S)[idx].astype(np.float64)
    t = ts.reshape(R, S)[idx].astype(np.float64)
    d = deltas.reshape(R, S)[idx].astype(np.float64)
    icw = np.cumsum(w, axis=1)
    wt = w * t
    r1 = np.sum(wt * (2.0 * icw - w), axis=1)
    r2 = np.sum(w * w * d, axis=1)
    r3 = np.sum(wt, axis=1)
    e_dist = 2.0 * LAM * (r1 + r2 / 6.0 - icw[:, -1] * r3)

    def rel(e, a):
        return np.max(np.abs(e - a)) / (np.max(np.abs(e)) + 1e-30)

    return max(rel(e_rgb, rgb_loss[idx].astype(np.float64)),
               rel(e_opa, opa_loss.reshape(R)[idx].astype(np.float64)),
               rel(e_dist, dist_loss[idx].astype(np.float64)))


def kernel(rgb_pred, rgb_gt, opacity, ws, deltas, ts, rays_a):
    global LAST_RESULTS
    if "nc" not in _NC_CACHE:
        _NC_CACHE["nc"] = _build_nc()
    nc = _NC_CACHE["nc"]

    rgb_pred = np.ascontiguousarray(rgb_pred, dtype=np.float32)
    rgb_gt = np.ascontiguousarray(rgb_gt, dtype=np.float32)
    opacity = np.ascontiguousarray(opacity, dtype=np.float32)
    ws = np.ascontiguousarray(ws, dtype=np.float32)
    deltas = np.ascontiguousarray(deltas, dtype=np.float32)
    ts = np.ascontiguousarray(ts, dtype=np.float32)

    ws_s = ws * np.float32(0.044721359549995794)  # sqrt(2*LAM)
    w5 = ws_s.reshape(NCORES, NT, 128, SEG, S)
    t5 = ts.reshape(NCORES, NT, 128, SEG, S)
    d5 = deltas.reshape(NCORES, NT, 128, SEG, S)
    smp = np.empty((NCORES, NT, 128, 3, SEG, S), dtype=np.float32)
    smp[:, :, :, 0] = w5
    smp[:, :, :, 1] = t5
    smp[:, :, :, 2] = d5

    rgbo = np.concatenate([
        rgb_pred.reshape(NCORES, 128, 384),
        rgb_gt.reshape(NCORES, 128, 384),
        opacity.reshape(NCORES, 128, 128),
    ], axis=2)

    in_maps = []
    for c in range(NCORES):
        in_maps.append({"smp": smp[c], "rgbo": rgbo[c]})

    for attempt in range(2):
        res = run_bass_kernel_spmd(nc, in_maps, list(range(NCORES)))
        LAST_RESULTS = res

        rgb_loss = np.concatenate(
            [res.results[c]["rgb_loss"].reshape(RPC, 3) for c in range(NCORES)],
            axis=0)
        opa_loss = np.concatenate(
            [res.results[c]["opa_loss"].reshape(RPC, 1) for c in range(NCORES)],
            axis=0)
        dist_loss = np.concatenate(
            [res.results[c]["dist_loss"].reshape(RPC) for c in range(NCORES)],
            axis=0)
        err = _selfcheck(rgb_pred, rgb_gt, opacity, ws, deltas, ts,
                         rgb_loss, opa_loss, dist_loss)
        if err < 1e-3:
            break
    return (rgb_loss, opa_loss, dist_loss)
